# revision 1
# baseline (speedup 1.0000x reference)
"""Single-head causal self-attention on 8 Trainium2 NeuronCores.

Problem: x[8, 4096, 1024], Wq/Wk/Wv[1024, 128] ->
  out[b] = softmax(causal((x[b] @ Wq) @ (x[b] @ Wk)^T / sqrt(128))) @ (x[b] @ Wv)

Sharding: data-parallel over batch -- each of the 8 cores handles one batch
element. Inputs are fed per-core as xT = x[b].T (layout prep on host) so the
contraction dim C lands on SBUF partitions.

Per-core kernel (T=4096, C=1024, HS=128), all matmuls in fp32r (full-rate
moving >= 256):
  Phase 1 (QKV): qT,kT [d=128, T] = sum_c Wq[c-chunk].T @ xT[c-chunk, :]
    vT likewise, then PE-transposed into v-natural [t, d] blocks.
  Phase 2 (attention), scores kept TRANSPOSED [kv, q] so that
    - PV needs no transposes: outT[d, q] += v_blk.T-free matmul
      (lhsT = v_blk [kv, d] natural, rhs = expT [kv, q]),
    - softmax denominator = partition-reduction done via a ones-vector matmul
      on a DVE-accumulated partial-sum tile.
    No max-subtraction: scaled scores are ~N(0,1), exp is safe in fp32.
    Causality: invalid 512-wide chunks skipped entirely; the diagonal
    128-block is masked with a precomputed upper-triangular 0/1 mask.
  Epilogue per q-group: PE-transpose outT -> out [q, d], scale rows by
    1/denominator, DMA out.
"""

import numpy as np
import ml_dtypes

import concourse.bass as bass
import concourse.tile as tile
from concourse import bacc, mybir
from concourse.bass_utils import run_bass_kernel_spmd

B, T, C, HS = 8, 4096, 1024, 128
P = 128
NCORES = 8
CCH = C // P            # 8 c-chunks
NT = T // P             # 32 t/kv blocks of 128
TG = T // 512           # 8 t-groups of 512 (phase 1)
QG = T // 1024          # 4 q-groups of 1024 (phase 2)
SCALE = float(HS) ** -0.5

f32 = mybir.dt.float32
f32r = mybir.dt.float32r
bf16 = mybir.dt.bfloat16
EXP = mybir.ActivationFunctionType.Exp

_NC = None


def build_program():
    nc = bacc.Bacc()
    xT = nc.declare_dram_parameter("xT", [C, T], bf16, isOutput=False)
    Wq = nc.declare_dram_parameter("Wq", [C, HS], bf16, isOutput=False)
    Wk = nc.declare_dram_parameter("Wk", [C, HS], bf16, isOutput=False)
    Wv = nc.declare_dram_parameter("Wv", [C, HS], bf16, isOutput=False)
    # host-provided constants: [ones(2) | identity(128) | trimask(128) | zeros(512)]
    aux = nc.declare_dram_parameter("aux", [P, 770], f32, isOutput=False)
    outT = nc.declare_dram_parameter("outT", [HS, T], f32, isOutput=True)
    rscratch = nc.dram_tensor("rscratch", [QG, 1024], f32)

    xT_r = xT[:].rearrange("(j p) t -> p j t", p=P)
    w_views = [w[:].rearrange("(j p) d -> p j d", p=P) for w in (Wq, Wk, Wv)]

    with tile.TileContext(nc) as tc:
        with (
            tc.tile_pool(name="consts", bufs=1) as consts,
            tc.tile_pool(name="big", bufs=1) as big,
        ):
            aux_sb = consts.tile([P, 770], f32r)
            nc.sync.dma_start(out=aux_sb[:], in_=aux[:].bitcast(f32r))
            ones = aux_sb[:, 0:2]
            ident = aux_sb[:, 2:130]
            trimask = aux_sb[:, 130:258]
            zeros = aux_sb[:, 258:770]

            trimask_b = consts.tile([P, P], bf16)
            nc.vector.tensor_copy(trimask_b[:], trimask)
            zeros_b = consts.tile([P, 512], bf16)
            nc.vector.tensor_copy(zeros_b[:], zeros)

            w_sb = [consts.tile([P, CCH, HS], bf16, tag=f"w{i}", name=f"w{i}")
                    for i in range(3)]
            for w_t, w_v in zip(w_sb, w_views):
                nc.sync.dma_start(out=w_t[:], in_=w_v)

            qT = big.tile([P, T], bf16, tag="qT")   # [d, t]
            kT = big.tile([P, T], bf16, tag="kT")   # [d, t]
            vS = big.tile([P, NT, HS], bf16, tag="vS")  # [t-in-block, block, d]

            # ---------------- Phase 1: QKV projections ----------------
            with (
                tc.tile_pool(name="xin", bufs=3) as xin,
                tc.tile_pool(name="vtp", bufs=2) as vtp,
                tc.tile_pool(name="ps_qkv", bufs=2, space="PSUM") as ps_qkv,
                tc.tile_pool(name="ps_tr", bufs=2, space="PSUM") as ps_tr,
            ):
                for tg in range(TG):
                    t0 = 512 * tg
                    xts = [xin.tile([P, 512], bf16, tag=f"xt{j}", name=f"xt{j}")
                           for j in range(CCH)]
                    for j in range(CCH):
                        nc.sync.dma_start(out=xts[j][:], in_=xT_r[:, j, t0:t0 + 512])

                    ps3 = [ps_qkv.tile([P, 512], f32, tag=f"ps{i}", name=f"ps{i}")
                           for i in range(3)]
                    for j in range(CCH):
                        for i in range(3):
                            nc.tensor.matmul(
                                ps3[i][:], lhsT=w_sb[i][:, j, :], rhs=xts[j][:],
                                start=(j == 0), stop=(j == CCH - 1),
                            )
                    nc.scalar.copy(qT[:, t0:t0 + 512], ps3[0][:])
                    nc.vector.tensor_copy(kT[:, t0:t0 + 512], ps3[1][:])
                    vt = vtp.tile([P, 512], f32r)
                    nc.vector.tensor_copy(vt[:], ps3[2][:])
                    for m in range(4):
                        tp = ps_tr.tile([P, P], f32r)
                        nc.tensor.transpose(tp[:], vt[:, 128 * m:128 * (m + 1)], ident)
                        eng = nc.scalar.copy if m % 2 == 0 else nc.vector.tensor_copy
                        eng(vS[:, 4 * tg + m, :], tp[:])

            # ---------------- Phase 2: causal attention ----------------
            with (
                tc.tile_pool(name="ptp", bufs=6) as ptp,
                tc.tile_pool(name="accp", bufs=2) as accp,
                tc.tile_pool(name="ocnp", bufs=2) as ocnp,
                tc.tile_pool(name="recipp", bufs=2) as recipp,
                tc.tile_pool(name="ps_s", bufs=2, space="PSUM") as ps_s,
                tc.tile_pool(name="ps_o", bufs=1, space="PSUM") as ps_o,
                tc.tile_pool(name="ps_dr", bufs=1, space="PSUM") as ps_dr,
            ):
                for g in range(QG):
                    q0 = 1024 * g
                    o_ps = ps_o.tile([P, 1024], f32)
                    acc = accp.tile([P, 1024], f32r, tag="acc", name="acc")
                    nkv = 8 * (g + 1)
                    for k in range(nkv):
                        vstart = max(0, 128 * k - q0)
                        s_ps = ps_s.tile([P, 1024], f32)
                        for c in range(2):
                            cq = 512 * c
                            lc = max(0, vstart - cq)
                            if lc >= 512:
                                continue  # chunk fully above diagonal
                            nc.tensor.matmul(
                                s_ps[:, cq + lc:cq + 512],
                                lhsT=kT[:, 128 * k:128 * (k + 1)],
                                rhs=qT[:, q0 + cq + lc:q0 + cq + 512],
                                start=True, stop=True,
                            )
                        pt = ptp.tile([P, 1024], bf16)
                        nc.scalar.activation(
                            pt[:, vstart:1024], s_ps[:, vstart:1024], EXP, scale=SCALE)
                        if k >= 8 * g:  # diagonal block: mask kv > q
                            ms = vstart - (vstart % 512)
                            if vstart % 512:
                                nc.vector.tensor_copy(
                                    pt[:, ms:vstart], zeros_b[:, 0:vstart - ms])
                            nc.vector.tensor_mul(
                                pt[:, vstart:vstart + 128],
                                pt[:, vstart:vstart + 128], trimask_b[:])
                        if k == 0:
                            nc.vector.tensor_copy(acc[:], pt[:])
                        else:
                            nc.vector.tensor_add(
                                acc[:, vstart:1024], acc[:, vstart:1024],
                                pt[:, vstart:1024])
                        for c in range(2):
                            cq = 512 * c
                            lc = max(0, vstart - cq)
                            if lc >= 512:
                                continue
                            last_k = 8 * g + 4 * c + 3
                            nc.tensor.matmul(
                                o_ps[:, cq + lc:cq + 512],
                                lhsT=vS[:, k, :], rhs=pt[:, cq + lc:cq + 512],
                                start=(k == 0), stop=(k == last_k),
                            )

                    # epilogue: free o_ps fast with a copy, then run the
                    # denominator/reciprocal chain off the PE critical path
                    ocu = ocnp.tile([P, 1024], f32, tag="ocu", name="ocu")
                    nc.vector.tensor_copy(ocu[:], o_ps[:])
                    dr_ps = ps_dr.tile([2, 1024], f32)
                    for c in range(2):
                        nc.tensor.matmul(
                            dr_ps[:, 512 * c:512 * (c + 1)],
                            lhsT=ones, rhs=acc[:, 512 * c:512 * (c + 1)],
                            start=True, stop=True,
                        )
                    recipT = recipp.tile([1, 1024], f32, tag="recipT", name="recipT")
                    rscr = recipp.tile([1, 1024], f32, tag="rscr", name="rscr")
                    nc.vector.reciprocal_approx_accurate(
                        recipT[:], dr_ps[0:1, :], rscr[:])
                    recipB = recipp.tile([P, 1024], f32, tag="recipB", name="recipB")
                    nc.sync.dma_start(out=rscratch[g:g + 1, :], in_=recipT[:])
                    rs = rscratch[g:g + 1, :]
                    rs_b = bass.AP(tensor=rs.tensor, offset=rs.offset,
                                   ap=[[0, P], rs.ap[-1]])
                    nc.sync.dma_start(out=recipB[:], in_=rs_b)
                    ocn = ocnp.tile([P, 1024], f32, tag="ocn", name="ocn")
                    nc.vector.tensor_mul(ocn[:], ocu[:], recipB[:])
                    nc.sync.dma_start(out=outT[:, q0:q0 + 1024], in_=ocn[:])

    nc.finalize()
    return nc


def _get_nc():
    global _NC
    if _NC is None:
        _NC = build_program()
    return _NC


def kernel(x, Wq, Wk, Wv):
    assert x.shape == (B, T, C) and Wq.shape == (C, HS)
    nc = _get_nc()
    x = np.asarray(x, dtype=np.float32)
    xb = x.astype(ml_dtypes.bfloat16)
    aux = np.zeros((P, 770), dtype=np.float32)
    aux[:, 0:2] = 1.0
    aux[:, 2:130] = np.eye(P, dtype=np.float32)
    iu = np.triu(np.ones((P, P), dtype=np.float32))  # 1 where kv <= q
    aux[:, 130:258] = iu
    in_maps = [
        {
            "xT": np.ascontiguousarray(xb[b].T),
            "Wq": np.asarray(Wq, dtype=np.float32).astype(ml_dtypes.bfloat16),
            "Wk": np.asarray(Wk, dtype=np.float32).astype(ml_dtypes.bfloat16),
            "Wv": np.asarray(Wv, dtype=np.float32).astype(ml_dtypes.bfloat16),
            "aux": aux,
        }
        for b in range(NCORES)
    ]
    res = run_bass_kernel_spmd(nc, in_maps, list(range(NCORES)))
    return np.stack([np.ascontiguousarray(res.results[b]["outT"].T)
                     for b in range(NCORES)])



# revision 2
# speedup vs baseline: 1.0996x; 1.0996x over previous
"""Single-head causal self-attention on 8 Trainium2 NeuronCores.

Problem: x[8, 4096, 1024], Wq/Wk/Wv[1024, 128] ->
  out[b] = softmax(causal((x[b] @ Wq) @ (x[b] @ Wk)^T / sqrt(128))) @ (x[b] @ Wv)

Sharding: data-parallel over batch -- each of the 8 cores handles one batch
element (xT = x[b].T fed per-core so the contraction dim C is on partitions).

Per-core kernel (T=4096, C=1024, HS=128), fp16 operands everywhere (more
mantissa than bf16 and unlocks DVE 2x mode for the fp16 accumulator adds):

  Phase 1 (QKV): x fully SBUF-resident (64KB/partition), DMA'd in 8 t-chunks
    so the first matmul starts ~2.5us. qT,kT [d,T] via W-stationary matmuls;
    v PE-transposed into natural [t,d] blocks.
  Phase 2 (attention), scores TRANSPOSED [kv, q], q-groups of 1024:
    - PSUM: 3-slot score ring (6 banks) + o_ps (2 banks).
    - exp on ScalarE, one slot per instruction, trimmed to the causal range.
    - causal masking of the diagonal 128-block via a PE accumulate-matmul
      (-60000*I @ strict-upper mask) added into the score PSUM -- exp then
      yields exact zeros, no DVE masking.
    - denominator: fp16 running acc += pt on DVE at 2x mode; per-group
      ones-matmul reduction.
    - epilogue entirely inside o_ps's own banks: ones-matmul denominator ->
      f32 reciprocal (DVE) -> fp16 cast -> PE broadcast matmul (ones x recip)
      -> one TT multiply. No DRAM round-trip.
  Scalar activation table preloaded with a dummy exp during the initial DMAs.
"""

import numpy as np

import concourse.bass as bass
import concourse.tile as tile
from concourse import bacc, mybir
from concourse.bass_utils import run_bass_kernel_spmd

B, T, C, HS = 8, 4096, 1024, 128
P = 128
NCORES = 8
CCH = C // P            # 8 c-chunks
NT = T // P             # 32 kv blocks of 128
TG = T // 512           # 8 t-groups of 512 (phase 1)
QG = T // 1024          # 4 q-groups of 1024 (phase 2)
SCALE = float(HS) ** -0.5
NEG = -60000.0          # large negative representable in fp16

f32 = mybir.dt.float32
f16 = mybir.dt.float16
EXP = mybir.ActivationFunctionType.Exp

_NC = None

# aux fp16 layout: [ones(130) | ident(128) | triU(128)]
AUX_W = 130 + 128 + 128


def build_aux() -> np.ndarray:
    aux = np.zeros((P, AUX_W), dtype=np.float16)
    aux[:, 0:130] = 1.0
    aux[:, 130:258] = np.eye(P, dtype=np.float16)
    # triU[c, q] = 1 where c > q  (kv > q within the diagonal 128-block)
    aux[:, 258:386] = np.tril(np.ones((P, P), dtype=np.float16), -1)
    return aux


def build_program():
    nc = bacc.Bacc()
    xT = nc.declare_dram_parameter("xT", [C, T], f16, isOutput=False)
    Wq = nc.declare_dram_parameter("Wq", [C, HS], f16, isOutput=False)
    Wk = nc.declare_dram_parameter("Wk", [C, HS], f16, isOutput=False)
    Wv = nc.declare_dram_parameter("Wv", [C, HS], f16, isOutput=False)
    aux = nc.declare_dram_parameter("aux", [P, AUX_W], f16, isOutput=False)
    outT = nc.declare_dram_parameter("outT", [HS, T], f32, isOutput=True)

    xT_r = xT[:].rearrange("(j p) t -> p j t", p=P)
    w_views = [w[:].rearrange("(j p) d -> p j d", p=P) for w in (Wq, Wk, Wv)]

    with tile.TileContext(nc) as tc:
        with (
            tc.tile_pool(name="consts", bufs=1) as consts,
            tc.tile_pool(name="big", bufs=1) as big,
        ):
            aux_sb = consts.tile([P, AUX_W], f16)
            nc.sync.dma_start(out=aux_sb[:], in_=aux[:])
            ones_col = aux_sb[:, 0:1]          # [128,1] dr lhsT
            ones_row = aux_sb[0:1, 0:128]      # [1,128] bcast lhsT
            ident = aux_sb[:, 130:258]         # transpose identity
            triU = aux_sb[:, 258:386]          # strict upper (kv>q) mask

            w_sb = [consts.tile([P, CCH, HS], f16, tag=f"w{i}", name=f"w{i}")
                    for i in range(3)]
            for i, w_v in enumerate(w_views):
                nc.sync.dma_start(out=w_sb[i][:], in_=w_v)

            # -60000 * I for PE-side causal masking (scaled from ident)
            identN = consts.tile([P, P], f16, tag="identN", name="identN")
            nc.vector.tensor_scalar_mul(identN[:], ident, NEG)

            # preload exp table while DMAs run
            warm = consts.tile([1, 2], f16, tag="warm", name="warm")
            nc.scalar.activation(warm[:], aux_sb[0:1, 0:2], EXP)

            x_sb = big.tile([P, CCH, T], f16, tag="x")
            qT = big.tile([P, T], f16, tag="qT")       # [d, t]
            kT = big.tile([P, T], f16, tag="kT")       # [d, t]
            vS = big.tile([P, NT, HS], f16, tag="vS")  # [t-in-block, blk, d]

            # ---------------- Phase 1: QKV projections ----------------
            with (
                tc.tile_pool(name="vtp", bufs=2) as vtp,
                tc.tile_pool(name="ps_qkv", bufs=2, space="PSUM") as ps_qkv,
                tc.tile_pool(name="ps_tr", bufs=2, space="PSUM") as ps_tr,
            ):
                for tg in range(TG):
                    t0 = 512 * tg
                    nc.sync.dma_start(out=x_sb[:, :, t0:t0 + 512],
                                      in_=xT_r[:, :, t0:t0 + 512])
                    ps3 = [ps_qkv.tile([P, 512], f32, tag=f"ps{i}",
                                       name=f"ps{i}") for i in range(3)]
                    for j in range(CCH):
                        for i in range(3):
                            nc.tensor.matmul(
                                ps3[i][:], lhsT=w_sb[i][:, j, :],
                                rhs=x_sb[:, j, t0:t0 + 512],
                                start=(j == 0), stop=(j == CCH - 1),
                            )
                    nc.vector.tensor_copy(qT[:, t0:t0 + 512], ps3[0][:])
                    nc.scalar.copy(kT[:, t0:t0 + 512], ps3[1][:])
                    vt = vtp.tile([P, 512], f16)
                    nc.vector.tensor_copy(vt[:], ps3[2][:])
                    for m in range(4):
                        tp = ps_tr.tile([P, P], f16)
                        nc.tensor.transpose(
                            tp[:], vt[:, 128 * m:128 * (m + 1)], ident)
                        eng = (nc.vector.tensor_copy if m % 2 == 0
                               else nc.scalar.copy)
                        eng(vS[:, 4 * tg + m, :], tp[:])

            # ---------------- Phase 2: causal attention ----------------
            with (
                tc.tile_pool(name="ptp", bufs=4) as ptp,
                tc.tile_pool(name="accp", bufs=2) as accp,
                tc.tile_pool(name="ocup", bufs=2) as ocup,
                tc.tile_pool(name="recipp", bufs=2) as recipp,
                tc.tile_pool(name="ocnp", bufs=2) as ocnp,
                tc.tile_pool(name="ring", bufs=3, space="PSUM") as ring,
                tc.tile_pool(name="ps_o", bufs=1, space="PSUM") as ps_o,
            ):
                for g in range(QG):
                    q0 = 1024 * g
                    o_ps = ps_o.tile([P, 1024], f32)
                    acc = accp.tile([P, 1024], f16, tag="acc", name="acc")
                    nkv = 8 * (g + 1)
                    for k in range(nkv):
                        va = max(0, 128 * k - q0)   # causal col start
                        s_ps = ring.tile([P, 1024], f32)
                        for c in range(2):
                            cq = 512 * c
                            lc = max(0, va - cq)
                            if lc >= 512:
                                continue
                            nc.tensor.matmul(
                                s_ps[:, cq + lc:cq + 512],
                                lhsT=kT[:, 128 * k:128 * (k + 1)],
                                rhs=qT[:, q0 + cq + lc:q0 + cq + 512],
                                start=True, stop=True,
                            )
                        if k >= 8 * g:
                            # diagonal block: add -60000 where kv > q
                            nc.tensor.matmul(
                                s_ps[:, va:va + 128],
                                lhsT=identN[:], rhs=triU,
                                start=False, stop=True,
                                skip_group_check=True,
                            )
                        pt = ptp.tile([P, 1024], f16)
                        nc.scalar.activation(
                            pt[:, va:1024], s_ps[:, va:1024], EXP, scale=SCALE)
                        if k == 0:
                            nc.vector.tensor_copy(acc[:], pt[:])
                        else:
                            nc.vector.tensor_add(
                                acc[:, va:1024], acc[:, va:1024],
                                pt[:, va:1024])
                        for c in range(2):
                            cq = 512 * c
                            lc = max(0, va - cq)
                            if lc >= 512:
                                continue
                            nc.tensor.matmul(
                                o_ps[:, cq + lc:cq + 512],
                                lhsT=vS[:, k, :],
                                rhs=pt[:, cq + lc:cq + 512],
                                start=(k == 0), stop=(k == 8 * g + 4 * c + 3),
                            )

                    # ---- epilogue (all inside o_ps's two banks) ----
                    ocu = ocup.tile([P, 1024], f32, tag="ocu", name="ocu")
                    nc.vector.tensor_copy(ocu[:], o_ps[:])
                    for c in range(2):
                        nc.tensor.matmul(
                            o_ps[0:1, 512 * c:512 * (c + 1)],
                            lhsT=ones_col, rhs=acc[:, 512 * c:512 * (c + 1)],
                            start=True, stop=True,
                        )
                    recipT = recipp.tile([1, 1024], f32, tag="recipT",
                                         name="recipT")
                    rscr = recipp.tile([1, 1024], f32, tag="rscr", name="rscr")
                    nc.vector.reciprocal_approx_accurate(
                        recipT[:], o_ps[0:1, 0:1024], rscr[:])
                    recipH = recipp.tile([1, 1024], f16, tag="recipH",
                                         name="recipH")
                    nc.vector.tensor_copy(recipH[:], recipT[:])
                    for c in range(2):
                        nc.tensor.matmul(
                            o_ps[:, 512 * c:512 * (c + 1)],
                            lhsT=ones_row, rhs=recipH[0:1, 512 * c:512 * (c + 1)],
                            start=True, stop=True,
                        )
                    ocn = ocnp.tile([P, 1024], f32, tag="ocn", name="ocn")
                    nc.vector.tensor_mul(ocn[:], ocu[:], o_ps[:])
                    nc.sync.dma_start(out=outT[:, q0:q0 + 1024], in_=ocn[:])

    nc.finalize()
    return nc


def _get_nc():
    global _NC
    if _NC is None:
        _NC = build_program()
    return _NC


def make_in_maps(x, Wq, Wk, Wv):
    xh = np.asarray(x, dtype=np.float32).astype(np.float16)
    wq = np.asarray(Wq, dtype=np.float32).astype(np.float16)
    wk = np.asarray(Wk, dtype=np.float32).astype(np.float16)
    wv = np.asarray(Wv, dtype=np.float32).astype(np.float16)
    aux = build_aux()
    return [
        {
            "xT": np.ascontiguousarray(xh[b].T),
            "Wq": wq, "Wk": wk, "Wv": wv,
            "aux": aux,
        }
        for b in range(NCORES)
    ]


def kernel(x, Wq, Wk, Wv):
    assert x.shape == (B, T, C) and Wq.shape == (C, HS)
    nc = _get_nc()
    in_maps = make_in_maps(x, Wq, Wk, Wv)
    res = run_bass_kernel_spmd(nc, in_maps, list(range(NCORES)))
    return np.stack([np.ascontiguousarray(res.results[b]["outT"].T)
                     for b in range(NCORES)])


# revision 9
# speedup vs baseline: 1.2035x; 1.0945x over previous
"""Single-head causal self-attention on 8 Trainium2 NeuronCores.

Problem: x[8, 4096, 1024], Wq/Wk/Wv[1024, 128] ->
  out[b] = softmax(causal((x[b] @ Wq) @ (x[b] @ Wk)^T / sqrt(128))) @ (x[b] @ Wv)

Sharding: data-parallel over batch -- each of the 8 cores handles one batch
element (xT = x[b].T fed per-core so the contraction dim C is on partitions).

Per-core kernel (T=4096, C=1024, HS=128), fp16 operands everywhere (more
mantissa than bf16 and unlocks DVE 2x mode for the fp16 accumulator adds):

  Phase 1 (QKV): x fully SBUF-resident (64KB/partition), DMA'd in 8 t-chunks
    so the first matmul starts ~2.5us. qT,kT [d,T] via W-stationary matmuls;
    v PE-transposed into natural [t,d] blocks.
  Phase 2 (attention), scores TRANSPOSED [kv, q], q-groups of 1024:
    - PSUM: 3-slot score ring (6 banks) + o_ps (2 banks).
    - exp on ScalarE, one slot per instruction, trimmed to the causal range.
    - causal masking of the diagonal 128-block via a PE accumulate-matmul
      (-60000*I @ strict-upper mask) added into the score PSUM -- exp then
      yields exact zeros, no DVE masking.
    - denominator: fp16 running acc += pt on DVE at 2x mode; per-group
      ones-matmul reduction.
    - epilogue entirely inside o_ps's own banks: ones-matmul denominator ->
      f32 reciprocal (DVE) -> fp16 cast -> PE broadcast matmul (ones x recip)
      -> one TT multiply. No DRAM round-trip.
  Scalar activation table preloaded with a dummy exp during the initial DMAs.
"""

import numpy as np

import concourse.bass as bass
import concourse.tile as tile
from concourse import bacc, mybir
from concourse.bass_utils import run_bass_kernel_spmd

B, T, C, HS = 8, 4096, 1024, 128
P = 128
NCORES = 8
CCH = C // P            # 8 c-chunks
NT = T // P             # 32 kv blocks of 128
TG = T // 512           # 8 t-groups of 512 (phase 1)
QG = T // 1024          # 4 q-groups of 1024 (phase 2)
SCALE = float(HS) ** -0.5
NEG = -60000.0          # large negative representable in fp16

f32 = mybir.dt.float32
f16 = mybir.dt.float16
EXP = mybir.ActivationFunctionType.Exp

_NC = None

# aux fp16 layout: [ones(130) | ident(128) | triU(128)]
AUX_W = 130 + 128 + 128


def build_aux() -> np.ndarray:
    aux = np.zeros((P, AUX_W), dtype=np.float16)
    aux[:, 0:130] = 1.0
    aux[:, 130:258] = np.eye(P, dtype=np.float16)
    # triU[c, q] = 1 where c > q  (kv > q within the diagonal 128-block)
    aux[:, 258:386] = np.tril(np.ones((P, P), dtype=np.float16), -1)
    return aux


def build_program():
    nc = bacc.Bacc()
    xT = nc.declare_dram_parameter("xT", [C, T], f16, isOutput=False)
    Wq = nc.declare_dram_parameter("Wq", [C, HS], f16, isOutput=False)
    Wk = nc.declare_dram_parameter("Wk", [C, HS], f16, isOutput=False)
    Wv = nc.declare_dram_parameter("Wv", [C, HS], f16, isOutput=False)
    aux = nc.declare_dram_parameter("aux", [P, AUX_W], f16, isOutput=False)
    outT = nc.declare_dram_parameter("outT", [HS, T], f32, isOutput=True)

    xT_r = xT[:].rearrange("(j p) t -> p j t", p=P)
    w_views = [w[:].rearrange("(j p) d -> p j d", p=P) for w in (Wq, Wk, Wv)]

    with tile.TileContext(nc) as tc:
        with (
            tc.tile_pool(name="consts", bufs=1) as consts,
            tc.tile_pool(name="big", bufs=1) as big,
        ):
            x_sb = big.tile([P, CCH, T], f16, tag="x")
            # first t-group split per c-chunk so matmuls can start ~5us in
            for j in range(CCH):
                nc.sync.dma_start(out=x_sb[:, j, 0:512],
                                  in_=xT_r[:, j, 0:512])
            aux_sb = consts.tile([P, AUX_W], f16)
            nc.sync.dma_start(out=aux_sb[:], in_=aux[:])
            ones_col = aux_sb[:, 0:1]          # [128,1] dr lhsT
            ones_row = aux_sb[0:1, 0:128]      # [1,128] bcast lhsT
            ident = aux_sb[:, 130:258]         # transpose identity
            triU = aux_sb[:, 258:386]          # strict upper (kv>q) mask

            w_sb = [consts.tile([P, CCH, HS], f16, tag=f"w{i}", name=f"w{i}")
                    for i in range(3)]
            for i, w_v in enumerate(w_views):
                nc.sync.dma_start(out=w_sb[i][:], in_=w_v)

            # -60000 * I for PE-side causal masking (scaled from ident)
            identN = consts.tile([P, P], f16, tag="identN", name="identN")
            nc.vector.tensor_scalar_mul(identN[:], ident, NEG)

            # preload exp table while DMAs run
            warm = consts.tile([1, 2], f16, tag="warm", name="warm")
            nc.scalar.activation(warm[:], aux_sb[0:1, 0:2], EXP)

            qT = big.tile([P, T], f16, tag="qT")       # [d, t]
            kT = big.tile([P, T], f16, tag="kT")       # [d, t]
            vS = big.tile([P, NT, HS], f16, tag="vS")  # [t-in-block, blk, d]

            # ---------------- Phase 1: QKV projections ----------------
            with (
                tc.tile_pool(name="vtp", bufs=2) as vtp,
                tc.tile_pool(name="ps_qkv", bufs=2, space="PSUM") as ps_qkv,
                tc.tile_pool(name="ps_tr", bufs=2, space="PSUM") as ps_tr,
            ):
                for tg in range(1, TG):
                    t0 = 512 * tg
                    nc.sync.dma_start(out=x_sb[:, :, t0:t0 + 512],
                                      in_=xT_r[:, :, t0:t0 + 512])
                for tg in range(TG):
                    t0 = 512 * tg
                    ps3 = [ps_qkv.tile([P, 512], f32, tag=f"ps{i}",
                                       name=f"ps{i}") for i in range(3)]
                    for j in range(CCH):
                        for i in range(3):
                            nc.tensor.matmul(
                                ps3[i][:], lhsT=w_sb[i][:, j, :],
                                rhs=x_sb[:, j, t0:t0 + 512],
                                start=(j == 0), stop=(j == CCH - 1),
                            )
                    nc.vector.tensor_copy(qT[:, t0:t0 + 512], ps3[0][:])
                    nc.scalar.copy(kT[:, t0:t0 + 512], ps3[1][:])
                    vt = vtp.tile([P, 512], f16)
                    nc.vector.tensor_copy(vt[:], ps3[2][:])
                    for m in range(4):
                        tp = ps_tr.tile([P, P], f16)
                        nc.tensor.transpose(
                            tp[:], vt[:, 128 * m:128 * (m + 1)], ident)
                        eng = (nc.vector.tensor_copy if m % 2 == 0
                               else nc.scalar.copy)
                        eng(vS[:, 4 * tg + m, :], tp[:])

            # ---------------- Phase 2: causal attention ----------------
            with (
                tc.tile_pool(name="ptp", bufs=6) as ptp,
                tc.tile_pool(name="accp", bufs=2) as accp,
                tc.tile_pool(name="ocup", bufs=2) as ocup,
                tc.tile_pool(name="recipp", bufs=2) as recipp,
                tc.tile_pool(name="ocnp", bufs=2) as ocnp,
                tc.tile_pool(name="ring", bufs=3, space="PSUM") as ring,
                tc.tile_pool(name="ps_o", bufs=1, space="PSUM") as ps_o,
            ):
                for g in range(QG):
                    q0 = 1024 * g
                    o_ps = ps_o.tile([P, 1024], f32)
                    acc = accp.tile([P, 1024], f16, tag="acc", name="acc")
                    nkv = 8 * (g + 1)
                    for k in range(nkv):
                        va = max(0, 128 * k - q0)   # causal col start
                        s_ps = ring.tile([P, 1024], f32, tag="s", name="s")
                        for c in range(2):
                            cq = 512 * c
                            lc = max(0, va - cq)
                            if lc >= 512:
                                continue
                            nc.tensor.matmul(
                                s_ps[:, cq + lc:cq + 512],
                                lhsT=kT[:, 128 * k:128 * (k + 1)],
                                rhs=qT[:, q0 + cq + lc:q0 + cq + 512],
                                start=True, stop=True,
                            )
                        if k >= 8 * g:
                            # diagonal block: add -60000 where kv > q
                            nc.tensor.matmul(
                                s_ps[:, va:va + 128],
                                lhsT=identN[:], rhs=triU,
                                start=False, stop=True,
                                skip_group_check=True,
                            )
                        pt = ptp.tile([P, 1024], f16)
                        nc.scalar.activation(
                            pt[:, va:1024], s_ps[:, va:1024], EXP, scale=SCALE)
                        if k == 0:
                            nc.vector.tensor_copy(acc[:], pt[:])
                        else:
                            nc.vector.tensor_add(
                                acc[:, va:1024], acc[:, va:1024],
                                pt[:, va:1024])
                        for c in range(2):
                            cq = 512 * c
                            lc = max(0, va - cq)
                            if lc >= 512:
                                continue
                            nc.tensor.matmul(
                                o_ps[:, cq + lc:cq + 512],
                                lhsT=vS[:, k, :],
                                rhs=pt[:, cq + lc:cq + 512],
                                start=(k == 0), stop=(k == 8 * g + 4 * c + 3),
                            )

                    # ---- epilogue: free o_ps ASAP (copy out), then run the
                    # denominator chain in a ring slot off the critical path
                    ocu = ocup.tile([P, 1024], f32, tag="ocu", name="ocu")
                    nc.scalar.copy(ocu[:], o_ps[:])
                    ep = ring.tile([P, 1024], f32, tag="s", name="s")
                    for c in range(2):
                        nc.tensor.matmul(
                            ep[0:1, 512 * c:512 * (c + 1)],
                            lhsT=ones_col, rhs=acc[:, 512 * c:512 * (c + 1)],
                            start=True, stop=True,
                        )
                    recipT = recipp.tile([1, 1024], f32, tag="recipT",
                                         name="recipT")
                    nc.vector.reciprocal_approx_fast(
                        recipT[:], ep[0:1, 0:1024])
                    recipH = recipp.tile([1, 1024], f16, tag="recipH",
                                         name="recipH")
                    nc.vector.tensor_copy(recipH[:], recipT[:])
                    for c in range(2):
                        nc.tensor.matmul(
                            ep[:, 512 * c:512 * (c + 1)],
                            lhsT=ones_row, rhs=recipH[0:1, 512 * c:512 * (c + 1)],
                            start=True, stop=True,
                        )
                    ocn = ocnp.tile([P, 1024], f32, tag="ocn", name="ocn")
                    nc.vector.tensor_mul(ocn[:], ocu[:], ep[:])
                    nc.sync.dma_start(out=outT[:, q0:q0 + 1024], in_=ocn[:])

    nc.finalize()
    return nc


def _get_nc():
    global _NC
    if _NC is None:
        _NC = build_program()
    return _NC


def make_in_maps(x, Wq, Wk, Wv):
    xh = np.asarray(x, dtype=np.float32).astype(np.float16)
    wq = np.asarray(Wq, dtype=np.float32).astype(np.float16)
    wk = np.asarray(Wk, dtype=np.float32).astype(np.float16)
    wv = np.asarray(Wv, dtype=np.float32).astype(np.float16)
    aux = build_aux()
    return [
        {
            "xT": np.ascontiguousarray(xh[b].T),
            "Wq": wq, "Wk": wk, "Wv": wv,
            "aux": aux,
        }
        for b in range(NCORES)
    ]


def kernel(x, Wq, Wk, Wv):
    assert x.shape == (B, T, C) and Wq.shape == (C, HS)
    nc = _get_nc()
    in_maps = make_in_maps(x, Wq, Wk, Wv)
    res = run_bass_kernel_spmd(nc, in_maps, list(range(NCORES)))
    return np.stack([np.ascontiguousarray(res.results[b]["outT"].T)
                     for b in range(NCORES)])
